# revision 1
# baseline (speedup 1.0000x reference)
"""Multi-head attention (B=2, H=8, S=2048, hd=16) on 8 Trainium2 NeuronCores.

Sharding: 16 (batch, head) attention groups -> 2 heads per core (cores 0-3:
batch 0, cores 4-7: batch 1).  Each core receives the (transposed) embeddings
for its batch, the 32 projection-weight columns for its two heads, and a
key-compacted copy of the embeddings (keys whose source mask is 0 contribute
exactly-zero softmax probability in fp32, so they are dropped; the compacted
set is padded to NK=1280 with -1000 additive-mask columns which also exp to
exactly 0).

Per head the kernel runs a two-pass softmax:
  pass A ([q,k] layout): S = (Q/4)K^T + mask via a 17-row contraction
    (16 dims + ones*mask row); DVE reduce_max(negate) gives -rowmax.
  pass B ([k,q] layout): S^T - rowmax via an 18-row contraction (16 dims +
    mask*ones + ones*(-rowmax)); ACT exp -> P^T in SBUF.
  ctx: P^T @ [V | 1] accumulated in PSUM with col-packed (tile_position)
    matmuls; the ones column yields the softmax denominator l. Final scale by
    1/l uses a gpsimd partition-broadcast + DVE multiply.

Output per core is a dense [32, 2048] (dim-major) tensor; the host scatters
columns back into the interleaved head layout (out[..., d*8+h] = ctx[d]).
"""

import numpy as np

S = 2048
E = 128
HD = 16
NK = 1280            # padded compacted key count (binomial(2048,1/2) + 11 sigma)
NKB = NK // 128      # 10 key blocks
NQB = S // 128       # 16 query blocks
NEG = -1000.0

_PROG = None


def _build_program():
    import concourse.mybir as mybir
    from concourse import bacc
    from concourse.tile import TileContext

    fp32 = mybir.dt.float32
    # float32r matmuls measured 1.2e-2 rel error on HW - too imprecise for
    # the score/ctx path; keep those exact fp32. The MAX pass is immune to
    # operand rounding (the bias cancels in normalization), so it runs on
    # real-f32r copies of Q/K at 1 cycle/row instead of 4.
    f32r = mybir.dt.float32
    f32rr = mybir.dt.float32r
    AF = mybir.ActivationFunctionType
    ALU = mybir.AluOpType
    AX = mybir.AxisListType

    nc = bacc.Bacc()

    xT = nc.declare_dram_parameter("xT", [E, S], f32r, isOutput=False)
    xkT = nc.declare_dram_parameter("xkT", [E, NK], f32r, isOutput=False)
    # weight columns padded to 48: head0 dims at 0:16, head1 dims at 32:48
    # (PSUM partition slices must start at 0/32/64/96)
    wq = nc.declare_dram_parameter("wq", [E, 48], f32r, isOutput=False)
    wk = nc.declare_dram_parameter("wk", [E, 48], f32r, isOutput=False)
    wv = nc.declare_dram_parameter("wv", [E, 48], f32r, isOutput=False)
    maskrow = nc.declare_dram_parameter("maskrow", [1, NK], f32r, isOutput=False)
    onesrow = nc.declare_dram_parameter("onesrow", [1, S], f32r, isOutput=False)
    onesr = nc.declare_dram_parameter("onesr", [1, S], f32rr, isOutput=False)
    maskr = nc.declare_dram_parameter("maskr", [1, NK], f32rr, isOutput=False)
    ident = nc.declare_dram_parameter("ident", [E, E], fp32, isOutput=False)
    out_d = nc.declare_dram_parameter("out", [2 * HD, S], fp32, isOutput=True)
    ldram = nc.dram_tensor("ldram", [2, S], fp32)

    with TileContext(nc) as tc:
        with (
            tc.tile_pool(name="consts", bufs=1) as cpool,
            tc.tile_pool(name="work", bufs=1) as wpool,
            tc.tile_pool(name="ptp", bufs=3) as ptpool,
            tc.tile_pool(name="stp", bufs=2, space="PSUM") as stpool,
            tc.tile_pool(name="ap", bufs=2, space="PSUM") as apool,
            tc.tile_pool(name="ctxp", bufs=2, space="PSUM") as ctxpool,
        ):
            # ---------------- constant loads ----------------
            xT_sb = cpool.tile([E, S], f32r, name="xT_sb")
            nc.sync.dma_start(out=xT_sb[:, :], in_=xT[:, :])
            xkT_sb = cpool.tile([E, NK], f32r, name="xkT_sb")
            nc.sync.dma_start(out=xkT_sb[:, :], in_=xkT[:, :])
            wq_sb = cpool.tile([E, 48], f32r, name="wq_sb")
            nc.sync.dma_start(out=wq_sb[:, :], in_=wq[:, :])
            wk_sb = cpool.tile([E, 48], f32r, name="wk_sb")
            nc.sync.dma_start(out=wk_sb[:, :], in_=wk[:, :])
            wv_sb = cpool.tile([E, 48], f32r, name="wv_sb")
            nc.sync.dma_start(out=wv_sb[:, :], in_=wv[:, :])
            ident_sb = cpool.tile([E, E], fp32, name="ident_sb")
            nc.sync.dma_start(out=ident_sb[:, :], in_=ident[:, :])

            # ---------------- persistent work tensors ----------------
            qt = [wpool.tile([18, S], f32r, name=f"qt{h}") for h in range(2)]
            kt = [wpool.tile([18, NK], f32r, name=f"kt{h}") for h in range(2)]
            qtr = [wpool.tile([17, S], f32rr, name=f"qtr{h}") for h in range(2)]
            ktr = [wpool.tile([17, NK], f32rr, name=f"ktr{h}") for h in range(2)]
            vv = [wpool.tile([128, NKB, HD + 1], f32r, name=f"vv{h}") for h in range(2)]
            negp = [wpool.tile([128, 3 * NQB], fp32, name=f"negp{h}") for h in range(2)]
            negc = [wpool.tile([128, NQB], fp32, name=f"negc{h}") for h in range(2)]
            nT_sb = [wpool.tile([NQB, 128], f32r, name=f"nT_sb{h}") for h in range(2)]
            ctxl = wpool.tile([49, S], fp32, name="ctxl")
            ldual = wpool.tile([33, S], fp32, name="ldual")
            linv = wpool.tile([33, S], fp32, name="linv")
            lbc = wpool.tile([48, S], fp32, name="lbc")
            out_sb = wpool.tile([64, S], fp32, name="out_sb")

            # ---------------- projections: QT, KT, V ----------------
            for half in range(2):
                qt_ps = stpool.tile([48, 1024], fp32, name="qt_ps", tag="st")
                for c in range(2):
                    nc.tensor.matmul(
                        qt_ps[:, 512 * c : 512 * (c + 1)],
                        lhsT=wq_sb[:, :],
                        rhs=xT_sb[:, 1024 * half + 512 * c : 1024 * half + 512 * (c + 1)],
                        start=True,
                        stop=True,
                    )
                for h in range(2):
                    # QT rows scaled by 1/sqrt(hd)=0.25; ones row below
                    nc.scalar.mul(
                        qt[h][0:16, 1024 * half : 1024 * (half + 1)],
                        qt_ps[32 * h : 32 * h + 16, :],
                        0.25,
                    )
                    nc.scalar.mul(
                        qtr[h][0:16, 1024 * half : 1024 * (half + 1)],
                        qt_ps[32 * h : 32 * h + 16, :],
                        0.25,
                    )
            for h in range(2):
                nc.sync.dma_start(out=qt[h][16:17, :], in_=onesrow[:, :])
                nc.sync.dma_start(out=qtr[h][16:17, :], in_=onesr[:, :])

            for o, n in ((0, 512), (512, 512), (1024, 256)):
                kt_ps = apool.tile([48, 512], fp32, name="kt_ps", tag="a")
                nc.tensor.matmul(
                    kt_ps[:, 0:n],
                    lhsT=wk_sb[:, :],
                    rhs=xkT_sb[:, o : o + n],
                    start=True,
                    stop=True,
                )
                for h in range(2):
                    nc.scalar.copy(
                        kt[h][0:16, o : o + n], kt_ps[32 * h : 32 * h + 16, 0:n]
                    )
                    nc.scalar.copy(
                        ktr[h][0:16, o : o + n], kt_ps[32 * h : 32 * h + 16, 0:n]
                    )
            for h in range(2):
                nc.sync.dma_start(out=kt[h][16:17, :], in_=maskrow[:, :])
                nc.sync.dma_start(out=ktr[h][16:17, :], in_=maskr[:, :])
                nc.sync.dma_start(out=kt[h][17:18, :], in_=onesrow[:, 0:NK])
                nc.sync.dma_start(
                    out=vv[h][:, :, HD : HD + 1],
                    in_=onesrow[0:1, 0:NKB].to_broadcast([128, NKB]),
                )

            # ---------------- phase helpers ----------------
            CH = ((0, 512), (512, 512), (1024, 256))  # pass-A k chunks

            def v_iter(kb):
                v_ps = apool.tile([128, 48], fp32, name="v_ps", tag="a")
                nc.tensor.matmul(
                    v_ps[:, :],
                    lhsT=xkT_sb[:, 128 * kb : 128 * (kb + 1)],
                    rhs=wv_sb[:, :],
                    start=True,
                    stop=True,
                )
                nc.vector.tensor_copy(
                    out=vv[0][:, kb, 0:HD], in_=v_ps[:, 0:16]
                )
                nc.vector.tensor_copy(
                    out=vv[1][:, kb, 0:HD], in_=v_ps[:, 32:48]
                )

            def a_iter(h, qb):
                lhs = qtr[h][0:17, 128 * qb : 128 * (qb + 1)]
                for ci, (o, n) in enumerate(CH):
                    sc = apool.tile([128, 512], fp32, name="sc", tag="a")
                    nc.tensor.matmul(
                        sc[:, 0:n],
                        lhsT=lhs,
                        rhs=ktr[h][0:17, o : o + n],
                        start=True,
                        stop=True,
                    )
                    nc.vector.tensor_reduce(
                        negp[h][:, 3 * qb + ci : 3 * qb + ci + 1],
                        sc[:, 0:n],
                        axis=AX.X,
                        op=ALU.max,
                        negate=True,
                    )

            def negm_assemble(h):
                nc.vector.tensor_reduce(
                    negc[h][:, :],
                    negp[h].rearrange("p (b t) -> p b t", t=3),
                    axis=AX.X,
                    op=ALU.min,
                )
                ntp = apool.tile([NQB, 128], fp32, name="ntp", tag="a")
                nc.tensor.transpose(ntp[:, :], negc[h][:, :], ident_sb[:, :])
                nc.vector.tensor_copy(out=nT_sb[h][:, :], in_=ntp[:, :])
                nc.sync.dma_start(
                    out=qt[h][17:18, :].rearrange("a (b f) -> a b f", b=NQB),
                    in_=nT_sb[h][:, :],
                )

            def b_iter(h, qh, kb, ctxc):
                st = stpool.tile([128, 1024], fp32, name="st", tag="st")
                lhs = kt[h][:, 128 * kb : 128 * (kb + 1)]
                for c in range(2):
                    nc.tensor.matmul(
                        st[:, 512 * c : 512 * (c + 1)],
                        lhsT=lhs,
                        rhs=qt[h][:, 1024 * qh + 512 * c : 1024 * qh + 512 * (c + 1)],
                        start=True,
                        stop=True,
                    )
                pt = ptpool.tile([128, 1024], f32r, name="pt", tag="pt")
                nc.scalar.activation(pt[:, :], st[:, :], AF.Exp)
                for c in range(2):
                    nc.tensor.matmul(
                        ctxc[c][0:17, :],
                        lhsT=vv[h][:, kb, :],
                        rhs=pt[:, 512 * c : 512 * (c + 1)],
                        start=(kb == 0),
                        stop=(kb == NKB - 1),
                    )

            def evac(h, qh, ctxc):
                for c in range(2):
                    nc.scalar.copy(
                        ctxl[
                            32 * h : 32 * h + 17,
                            1024 * qh + 512 * c : 1024 * qh + 512 * (c + 1),
                        ],
                        ctxc[c][0:17, :],
                    )

            def b_half(h, qh):
                ctxc = [
                    ctxpool.tile([17, 512], fp32, name=f"ctx{c}", tag="ctx")
                    for c in range(2)
                ]
                return ctxc

            # ---------------- schedule ----------------
            # A(h0), with V projections interleaved
            for qb in range(NQB):
                a_iter(0, qb)
                if qb < NKB:
                    v_iter(qb)
            negm_assemble(0)

            # B(h0) (2 q-halves x NKB) overlapped with A(h1)
            ai = 0
            for qh in range(2):
                ctxc = b_half(0, qh)
                for kb in range(NKB):
                    b_iter(0, qh, kb, ctxc)
                    if ai < NQB and (kb % 2 == 0 or qh == 1):
                        a_iter(1, ai)
                        ai += 1
                evac(0, qh, ctxc)
            while ai < NQB:
                a_iter(1, ai)
                ai += 1
            negm_assemble(1)

            # B(h1)
            for qh in range(2):
                ctxc = b_half(1, qh)
                for kb in range(NKB):
                    b_iter(1, qh, kb, ctxc)
                evac(1, qh, ctxc)

            # ---------------- finals ----------------
            for h in range(2):
                nc.sync.dma_start(
                    out=ldual[32 * h : 32 * h + 1, :],
                    in_=ctxl[32 * h + 16 : 32 * h + 17, :],
                )
                nc.vector.reciprocal(
                    linv[32 * h : 32 * h + 1, :], ldual[32 * h : 32 * h + 1, :]
                )
                nc.sync.dma_start(
                    out=ldram[h : h + 1, :], in_=linv[32 * h : 32 * h + 1, :]
                )
                nc.sync.dma_start(
                    out=lbc[32 * h : 32 * h + 16, :],
                    in_=ldram[h : h + 1, :].to_broadcast([HD, S]),
                )
                nc.vector.tensor_tensor(
                    out=out_sb[32 * h : 32 * h + 16, :],
                    in0=ctxl[32 * h : 32 * h + 16, :],
                    in1=lbc[32 * h : 32 * h + 16, :],
                    op=mybir.AluOpType.mult,
                )
            for h in range(2):
                nc.sync.dma_start(
                    out=out_d[16 * h : 16 * h + 16, :],
                    in_=out_sb[32 * h : 32 * h + 16, :],
                )

    nc.finalize()
    return nc


def _prep_core_inputs(x, msk_add_full, w_query, w_key, w_value):
    """Build the 8 per-core input maps from full inputs."""
    B = x.shape[0]
    in_maps = []
    onesrow = np.ones((1, S), dtype=np.float32)
    identm = np.eye(E, dtype=np.float32)
    per_batch = []
    for b in range(B):
        keep = np.flatnonzero(msk_add_full[b] == 0.0)
        nk = len(keep)
        assert 0 < nk <= NK, f"compacted key count {nk} out of range"
        xk = np.zeros((NK, E), dtype=np.float32)
        xk[:nk] = x[b][keep]
        maskrow = np.full((1, NK), NEG, dtype=np.float32)
        maskrow[0, :nk] = 0.0
        xTb = np.ascontiguousarray(x[b].T)
        xkTb = np.ascontiguousarray(xk.T)
        per_batch.append((xTb, xkTb, maskrow))
    for c in range(8):
        b = c // 4
        h0 = 2 * (c % 4)
        xTb, xkTb, maskrow = per_batch[b]
        def _pad48(w):
            wc = np.zeros((E, 48), dtype=np.float32)
            wc[:, 0:16] = w[:, h0::8]
            wc[:, 32:48] = w[:, h0 + 1 :: 8]
            return wc

        wq_c = _pad48(w_query)
        wk_c = _pad48(w_key)
        wv_c = _pad48(w_value)
        in_maps.append(
            {
                "xT": xTb,
                "xkT": xkTb,
                "wq": wq_c,
                "wk": wk_c,
                "wv": wv_c,
                "maskrow": maskrow,
                "maskr": maskrow,
                "onesrow": onesrow,
                "onesr": onesrow,
                "ident": identm,
            }
        )
    return in_maps


def kernel(
    input_embeddings,
    token_attention_masks_source,
    token_attention_masks_target,
    masked,
    w_query,
    w_key,
    w_value,
):
    global _PROG
    x = np.asarray(input_embeddings, dtype=np.float32)
    msk = np.asarray(token_attention_masks_source)
    wq_f = np.asarray(w_query, dtype=np.float32)
    wk_f = np.asarray(w_key, dtype=np.float32)
    wv_f = np.asarray(w_value, dtype=np.float32)
    assert int(np.asarray(masked)) == 0, "only the encoder (masked=0) path is supported"
    B = x.shape[0]
    assert x.shape == (2, S, E)

    msk_add = np.where(msk == 0, np.float32(NEG), np.float32(0.0))
    in_maps = _prep_core_inputs(x, msk_add, wq_f, wk_f, wv_f)

    if _PROG is None:
        _PROG = _build_program()
    nc = _PROG

    from concourse.bass_utils import run_bass_kernel_spmd

    res = run_bass_kernel_spmd(nc, in_maps, list(range(8)))

    out = np.empty((B, S, E), dtype=np.float32)
    for c in range(8):
        b = c // 4
        h0 = 2 * (c % 4)
        o = res.results[c]["out"]  # [32, 2048]
        out[b][:, h0::8] = o[0:16, :].T
        out[b][:, h0 + 1 :: 8] = o[16:32, :].T
    return out



# revision 19
# speedup vs baseline: 1.8471x; 1.8471x over previous
"""Multi-head attention (B=2, H=8, S=2048, hd=16) on 8 Trainium2 NeuronCores.

Sharding: 16 (batch, head) attention groups -> 2 heads per core (cores 0-3:
batch 0, cores 4-7: batch 1).  Each core receives the (transposed) embeddings
for its batch, the 32 projection-weight columns for its two heads, and a
key-compacted copy of the embeddings (keys whose source mask is 0 contribute
exactly-zero softmax probability in fp32, so they are dropped; the compacted
set is padded with -1000 additive-mask columns which also exp to exactly 0).

All matmuls stream float32r (1 PE cycle/column vs 4 for exact fp32); the
bf16-level operand rounding keeps end-to-end error ~1e-2, inside the 2e-2
gate.  Per head the kernel runs a two-pass softmax:

  pass A ([q,k] layout, keys padded to NKA=1280 so every PSUM chunk is
    >=256 columns): S = (Q/4)K^T + mask via a 17-row contraction; a single
    fused DVE tensor_tensor_reduce per q-block computes
    min(-max(sc_lo, sc_hi)) = -rowmax in one 640-element pass.
  pass B ([k,q] layout, NK=1152): S^T - rowmax via an 18-row contraction
    (16 dims + mask*ones + ones*(-rowmax)); ACT exp -> P^T; ctx accumulates
    P^T @ [V | 1] in PSUM, where the ones column yields the softmax
    denominator l.  Final scale by 1/l uses a DMA partition-broadcast and a
    Pool-engine multiply.

The B pipeline is software-pipelined (st/exp of iteration i+1 issued before
ctx of iteration i) and pass A of later (head, q-half) groups is injected
one q-block per B iteration so the PE never idles waiting for DVE reduces.
PSUM: one shared 2-buf pool (3 banks per buf) rotates st [128,1024]f32 and
sc [128,1280]f32 tiles; ctx [17,1024] holds the remaining 2 banks.

Output per core is a dense [32, 2048] (dim-major) tensor; the host scatters
columns back into the interleaved head layout (out[..., d*8+h] = ctx[d]).
"""

import numpy as np

S = 2048
E = 128
HD = 16
NK = 1152            # pass-B padded compacted key count (9 blocks of 128)
NKA = 1280           # pass-A padded key count (chunks 512/512/256, all >=256)
NKB = NK // 128
NQB = S // 128
NEG = -1000.0

_PROG = None


def _build_program():
    import concourse.mybir as mybir
    from concourse import bacc
    from concourse.tile import TileContext

    fp32 = mybir.dt.float32
    f32r = mybir.dt.float32r
    AF = mybir.ActivationFunctionType
    ALU = mybir.AluOpType
    AX = mybir.AxisListType

    nc = bacc.Bacc()

    xT = nc.declare_dram_parameter("xT", [E, S], f32r, isOutput=False)
    xkT = nc.declare_dram_parameter("xkT", [E, NKA], f32r, isOutput=False)
    # weight columns padded to 48: head0 dims at 0:16, head1 dims at 32:48
    # (PSUM partition slices must start at 0/32/64/96); wq pre-scaled by 0.25
    wq = nc.declare_dram_parameter("wq", [E, 48], f32r, isOutput=False)
    wk = nc.declare_dram_parameter("wk", [E, 48], f32r, isOutput=False)
    wv = nc.declare_dram_parameter("wv", [E, 48], f32r, isOutput=False)
    maskrow = nc.declare_dram_parameter("maskrow", [1, NKA], f32r, isOutput=False)
    onesr = nc.declare_dram_parameter("onesr", [1, S], f32r, isOutput=False)
    ident = nc.declare_dram_parameter("ident", [E, E], fp32, isOutput=False)
    out_d = nc.declare_dram_parameter("out", [2 * HD, S], fp32, isOutput=True)
    ldram = nc.dram_tensor("ldram", [2, S], fp32)

    with TileContext(nc) as tc:
        with (
            tc.tile_pool(name="consts", bufs=1) as cpool,
            tc.tile_pool(name="work", bufs=1) as wpool,
            tc.tile_pool(name="ptp", bufs=3) as ptpool,
            tc.tile_pool(name="big", bufs=2, space="PSUM") as bigpool,
            tc.tile_pool(name="ctxp", bufs=2, space="PSUM") as ctxpool,
        ):
            # ---------------- constant loads ----------------
            xkT_sb = cpool.tile([E, NKA], f32r, name="xkT_sb")
            for o, n in ((0, 512), (512, 512), (1024, 256)):
                nc.sync.dma_start(out=xkT_sb[:, o : o + n], in_=xkT[:, o : o + n])
            xT_sb = cpool.tile([E, S], f32r, name="xT_sb")
            for o in range(0, S, 512):
                nc.sync.dma_start(out=xT_sb[:, o : o + 512], in_=xT[:, o : o + 512])
            wq_sb = cpool.tile([E, 48], f32r, name="wq_sb")
            nc.sync.dma_start(out=wq_sb[:, :], in_=wq[:, :])
            wk_sb = cpool.tile([E, 48], f32r, name="wk_sb")
            nc.sync.dma_start(out=wk_sb[:, :], in_=wk[:, :])
            wv_sb = cpool.tile([E, 48], f32r, name="wv_sb")
            nc.sync.dma_start(out=wv_sb[:, :], in_=wv[:, :])
            ident_sb = cpool.tile([E, E], fp32, name="ident_sb")
            nc.sync.dma_start(out=ident_sb[:, :], in_=ident[:, :])

            # ---------------- persistent work tensors ----------------
            # Both heads stacked in one tile at partition offsets 0 / 32 so a
            # single PSUM-evac copy serves both heads (cost is free-dim only).
            # qtall rows per head h (base b=32h): b..b+15 q-dims (x0.25),
            #   b+16 ones, b+17 -rowmax.
            # ktall rows: b..b+15 k-dims, b+16 mask, b+17 ones.
            qtall = wpool.tile([50, S], f32r, name="qtall")
            ktall = wpool.tile([50, NKA], f32r, name="ktall")
            vv = [wpool.tile([128, NKB, HD + 1], f32r, name=f"vv{h}") for h in range(2)]
            negc = [wpool.tile([128, NQB], fp32, name=f"negc{h}") for h in range(2)]
            nT_sb = [wpool.tile([8, 2, 128], f32r, name=f"nT_sb{h}") for h in range(2)]
            ctxl = wpool.tile([49, S], fp32, name="ctxl")
            l128 = wpool.tile([128, 2, HD], fp32, name="l128")
            linv128 = wpool.tile([128, 2, HD], fp32, name="linv128")
            lbc = wpool.tile([48, S], fp32, name="lbc")
            out_sb = wpool.tile([64, S], fp32, name="out_sb")

            # ---------------- projections: KT, QT, V ----------------
            # One [48,n] copy covers both heads' rows; the junk it writes to
            # rows 16:32 is overwritten by nothing (rows 16/48 get the mask /
            # ones DMAs afterwards, rows 18:32 are never read).
            for o, n in ((0, 512), (512, 512), (1024, 256)):
                kt_ps = bigpool.tile([48, 512], fp32, name="kt_ps", tag="big")
                nc.tensor.matmul(
                    kt_ps[:, 0:n],
                    lhsT=wk_sb[:, :],
                    rhs=xkT_sb[:, o : o + n],
                    start=True,
                    stop=True,
                )
                nc.scalar.copy(ktall[0:48, o : o + n], kt_ps[:, 0:n])
            for h in range(2):
                nc.sync.dma_start(
                    out=ktall[32 * h + 16 : 32 * h + 17, :], in_=maskrow[:, :]
                )
                nc.sync.dma_start(
                    out=ktall[32 * h + 17 : 32 * h + 18, 0:NK], in_=onesr[:, 0:NK]
                )

            for half in range(2):
                qt_ps = bigpool.tile([48, 1024], fp32, name="qt_ps", tag="big")
                for c in range(2):
                    nc.tensor.matmul(
                        qt_ps[:, 512 * c : 512 * (c + 1)],
                        lhsT=wq_sb[:, :],
                        rhs=xT_sb[:, 1024 * half + 512 * c : 1024 * half + 512 * (c + 1)],
                        start=True,
                        stop=True,
                    )
                nc.scalar.copy(
                    qtall[0:48, 1024 * half : 1024 * (half + 1)], qt_ps[:, :]
                )
            for h in range(2):
                nc.sync.dma_start(
                    out=qtall[32 * h + 16 : 32 * h + 17, :], in_=onesr[:, :]
                )

            # V projection: all key blocks into one PSUM tile, then one
            # strided copy per head; ones column via DMA broadcast.
            v_ps = bigpool.tile([128, NKB, 48], fp32, name="v_ps", tag="big")
            for kb in range(NKB):
                nc.tensor.matmul(
                    v_ps[:, kb, :],
                    lhsT=xkT_sb[:, 128 * kb : 128 * (kb + 1)],
                    rhs=wv_sb[:, :],
                    start=True,
                    stop=True,
                )
            for h in range(2):
                nc.scalar.copy(vv[h][:, :, 0:HD], v_ps[:, :, 32 * h : 32 * h + 16])
                nc.sync.dma_start(
                    out=vv[h][:, :, HD : HD + 1],
                    in_=onesr[0:1, 0:NKB].to_broadcast([128, NKB]),
                )

            # ---------------- pass A: -rowmax per q-block ----------------
            def a_iter(h, qb):
                sc = bigpool.tile([128, NKA], fp32, name="sc", tag="big")
                lhs = qtall[32 * h : 32 * h + 17, 128 * qb : 128 * (qb + 1)]
                for o, n in ((0, 512), (512, 512), (1024, 256)):
                    nc.tensor.matmul(
                        sc[:, o : o + n],
                        lhsT=lhs,
                        rhs=ktall[32 * h : 32 * h + 17, o : o + n],
                        start=True,
                        stop=True,
                    )
                # pad columns NK: are all -1000 and rowmax >= 0 here, so the
                # reduce only needs the first NK columns
                nc.vector.tensor_reduce(
                    negc[h][:, qb : qb + 1],
                    sc[:, 0:NK],
                    axis=AX.X,
                    op=ALU.max,
                    negate=True,
                )

            def negm_assemble(h, qh):
                ntp = bigpool.tile([8, 128], fp32, name="ntp", tag="big")
                nc.tensor.transpose(
                    ntp[:, :], negc[h][:, 8 * qh : 8 * (qh + 1)], ident_sb[:, :]
                )
                nc.vector.tensor_copy(out=nT_sb[h][:, qh, :], in_=ntp[:, :])
                nc.sync.dma_start(
                    out=qtall[
                        32 * h + 17 : 32 * h + 18, 1024 * qh : 1024 * (qh + 1)
                    ].rearrange("a (b f) -> a b f", b=8),
                    in_=nT_sb[h][:, qh, :],
                )

            # ---------------- pass B pipeline pieces ----------------
            def st_exp(h, qh, kb):
                st = bigpool.tile([128, 1024], fp32, name="st", tag="big")
                lhs = ktall[32 * h : 32 * h + 18, 128 * kb : 128 * (kb + 1)]
                for c in range(2):
                    nc.tensor.matmul(
                        st[:, 512 * c : 512 * (c + 1)],
                        lhsT=lhs,
                        rhs=qtall[
                            32 * h : 32 * h + 18,
                            1024 * qh + 512 * c : 1024 * qh + 512 * (c + 1),
                        ],
                        start=True,
                        stop=True,
                    )
                pt = ptpool.tile([128, 1024], f32r, name="pt", tag="pt")
                nc.scalar.activation(pt[:, :], st[:, :], AF.Exp)
                return pt

            def ctx_acc(h, kb, ctxc, pt):
                for c in range(2):
                    nc.tensor.matmul(
                        ctxc[c][0:17, :],
                        lhsT=vv[h][:, kb, :],
                        rhs=pt[:, 512 * c : 512 * (c + 1)],
                        start=(kb == 0),
                        stop=(kb == NKB - 1),
                    )

            def evac(h, qh, ctxc):
                for c in range(2):
                    nc.scalar.copy(
                        ctxl[
                            32 * h : 32 * h + 17,
                            1024 * qh + 512 * c : 1024 * qh + 512 * (c + 1),
                        ],
                        ctxc[c][0:17, :],
                    )

            def finals(h):
                # l row -> [128,16] layout so the reciprocal runs wide on DVE
                nc.sync.dma_start(
                    out=l128[:, h, :],
                    in_=ctxl[32 * h + 16 : 32 * h + 17, :].rearrange(
                        "a (b f) -> a b f", b=128
                    ),
                )
                nc.vector.reciprocal(linv128[:, h, :], l128[:, h, :])
                nc.sync.dma_start(
                    out=ldram[h : h + 1, :].rearrange("a (b f) -> a b f", b=128),
                    in_=linv128[:, h, :],
                )
                nc.sync.dma_start(
                    out=lbc[32 * h : 32 * h + 16, :],
                    in_=ldram[h : h + 1, :].to_broadcast([HD, S]),
                )
                eng = nc.gpsimd if h == 0 else nc.vector
                eng.tensor_tensor(
                    out=out_sb[32 * h : 32 * h + 16, :],
                    in0=ctxl[32 * h : 32 * h + 16, :],
                    in1=lbc[32 * h : 32 * h + 16, :],
                    op=ALU.mult,
                )
                nc.sync.dma_start(
                    out=out_d[16 * h : 16 * h + 16, :],
                    in_=out_sb[32 * h : 32 * h + 16, :],
                )

            # ---------------- schedule ----------------
            # Prologue: pass A for (h0, qh0) so the B pipeline can start.
            for qb in range(8):
                a_iter(0, qb)
            negm_assemble(0, 0)

            # Remaining pass-A q-blocks, injected one per B iteration.
            inject = [(0, qb) for qb in range(8, 16)] + [
                (1, qb) for qb in range(16)
            ]
            # negm assemblies fire once their 8 q-blocks have been issued
            negm_after = {8: (0, 1), 16: (1, 0), 24: (1, 1)}

            halves = [(h, qh) for h in range(2) for qh in range(2)]
            prev = None  # (h, qh, kb, ctxc, pt) awaiting its ctx matmul
            ii = 0  # injection cursor
            for h, qh in halves:
                ctxc = [
                    ctxpool.tile([17, 512], fp32, name=f"ctx{c}", tag="ctx")
                    for c in range(2)
                ]
                for kb in range(NKB):
                    pt = st_exp(h, qh, kb)
                    if ii in negm_after:
                        negm_assemble(*negm_after[ii])
                        del negm_after[ii]
                    if ii < len(inject):
                        a_iter(*inject[ii])
                        ii += 1
                    if prev is not None:
                        ph, pqh, pkb, pctxc, ppt = prev
                        ctx_acc(ph, pkb, pctxc, ppt)
                        if pkb == NKB - 1:
                            evac(ph, pqh, pctxc)
                            if pqh == 1:
                                finals(ph)
                    prev = (h, qh, kb, ctxc, pt)
            ph, pqh, pkb, pctxc, ppt = prev
            ctx_acc(ph, pkb, pctxc, ppt)
            evac(ph, pqh, pctxc)
            finals(1)

    nc.finalize()
    return nc


def _prep_core_inputs(x, msk_add_full, w_query, w_key, w_value):
    """Build the 8 per-core input maps from full inputs."""
    B = x.shape[0]
    in_maps = []
    onesrow = np.ones((1, S), dtype=np.float32)
    identm = np.eye(E, dtype=np.float32)
    per_batch = []
    for b in range(B):
        keep = np.flatnonzero(msk_add_full[b] == 0.0)
        nk = len(keep)
        assert 0 < nk <= NK, f"compacted key count {nk} out of range"
        xk = np.zeros((NKA, E), dtype=np.float32)
        xk[:nk] = x[b][keep]
        maskrow = np.full((1, NKA), NEG, dtype=np.float32)
        maskrow[0, :nk] = 0.0
        xTb = np.ascontiguousarray(x[b].T)
        xkTb = np.ascontiguousarray(xk.T)
        per_batch.append((xTb, xkTb, maskrow))
    for c in range(8):
        b = c // 4
        h0 = 2 * (c % 4)
        xTb, xkTb, maskrow = per_batch[b]

        def _pad48(w, scale=1.0):
            wc = np.zeros((E, 48), dtype=np.float32)
            wc[:, 0:16] = w[:, h0::8] * scale
            wc[:, 32:48] = w[:, h0 + 1 :: 8] * scale
            return wc

        in_maps.append(
            {
                "xT": xTb,
                "xkT": xkTb,
                "wq": _pad48(w_query, 0.25),
                "wk": _pad48(w_key),
                "wv": _pad48(w_value),
                "maskrow": maskrow,
                "onesr": onesrow,
                "ident": identm,
            }
        )
    return in_maps


def kernel(
    input_embeddings,
    token_attention_masks_source,
    token_attention_masks_target,
    masked,
    w_query,
    w_key,
    w_value,
):
    global _PROG
    x = np.asarray(input_embeddings, dtype=np.float32)
    msk = np.asarray(token_attention_masks_source)
    wq_f = np.asarray(w_query, dtype=np.float32)
    wk_f = np.asarray(w_key, dtype=np.float32)
    wv_f = np.asarray(w_value, dtype=np.float32)
    assert int(np.asarray(masked)) == 0, "only the encoder (masked=0) path is supported"
    B = x.shape[0]
    assert x.shape == (2, S, E)

    msk_add = np.where(msk == 0, np.float32(NEG), np.float32(0.0))
    in_maps = _prep_core_inputs(x, msk_add, wq_f, wk_f, wv_f)

    if _PROG is None:
        _PROG = _build_program()
    nc = _PROG

    from concourse.bass_utils import run_bass_kernel_spmd

    res = run_bass_kernel_spmd(nc, in_maps, list(range(8)))

    out = np.empty((B, S, E), dtype=np.float32)
    for c in range(8):
        b = c // 4
        h0 = 2 * (c % 4)
        o = res.results[c]["out"]  # [32, 2048]
        out[b][:, h0::8] = o[0:16, :].T
        out[b][:, h0 + 1 :: 8] = o[16:32, :].T
    return out
